# revision 16
# baseline (speedup 1.0000x reference)
"""CPRINT4Linear on 8 TRN2 NeuronCores.

out[M,N] = gather_cols(x)[M,K] @ dequant_int4(w_packed)[K,N] + bias

Strategy:
- Shard M (=B*S=8192) across 8 cores (data parallel): core c computes out rows
  [1024c, 1024(c+1)). No collectives; host concatenates the shards.
- Host folds the col_indices gather + transpose + nibble-deinterleave into one
  permuted transpose of x: xT[kk, m] = x[m, perm[kk]].  The k-order is chosen
  so that dequant block b (packed rows [128b, 128b+128)) yields k-tile 2b as
  the low nibbles and k-tile 2b+1 as the high nibbles, with scale group 2b on
  partitions 0..63 and group 2b+1 on partitions 64..127 of both k-tiles.
- Per core, per 512-wide n-chunk of the 11008 out columns: dequant per block
  (plain 64KB wp load -> two DVE nibble extracts -> one ACT debias/cast ->
  one DVE scale-mul into float32r), then matmuls with the dequantized weight
  tile as the STATIONARY operand and 512-wide xT slices as the MOVING operand
  (halves weight-load overhead), accumulating out^T [128n, 512m] tiles over
  the 32 k-tiles in all 8 PSUM banks.  PSUM -> SBUF copyback alternates
  DVE/ACT, out^T DMA'd to DRAM; host transposes at unshard.
- float32r matmul: full PE rate (1 cycle/row, 4x faster than fp32) at ~1.5e-4
  relative error; x and w are DMA'd/computed directly into float32r tiles.
- Software-pipelined flat schedule: dequant blocks produced `lookahead` blocks
  ahead of their matmul consumption; xT tiles loaded just-in-time during the
  first chunk and resident in SBUF (128KB/partition) thereafter.
- DMA queues: wp on SP HWDGE, scales/xT/out on ACT HWDGE.
- Bias is added on host during unshard (numpy broadcast, exact fp32).

Measured (axon TRN2, repeat-delta method): ~1.34 ms/pass vs a ~1.17 ms pure
PE-streaming floor (5504 matmuls x 512 cols @ 2.4 GHz); rel err 1.49e-4.
"""
import numpy as np

import concourse.bacc as bacc
import concourse.mybir as mybir
from concourse.tile import TileContext
from concourse.bass_utils import run_bass_kernel_spmd

B, S, K, N = 4, 2048, 4096, 11008
M = B * S
NCORES = 8
MC = M // NCORES            # 1024 rows per core
G = K // 128                # 32 k-tiles == dequant groups
N_CHUNKS = [512] * 21 + [256]
MT = MC // 128              # 8 m-tiles per core

F32 = mybir.dt.float32
F32R = mybir.dt.float32r
U8 = mybir.dt.uint8

# k-order: block b = packed rows [128b, 128b+128) = k-tiles 2b (low nibbles)
# and 2b+1 (high nibbles) = scale groups 2b (partitions 0..63) and 2b+1
# (partitions 64..127).  kk = 256b + 128t + p  ->  w_int row j = 256b + 2p + t.
_b = np.arange(G // 2)[:, None, None]
_t = np.arange(2)[None, :, None]
_p = np.arange(128)[None, None, :]
J_ORDER = (256 * _b + 2 * _p + _t).reshape(-1)

TRACE = False
LAST_RESULTS = None
_CACHED_NC = None


ORIENT_T = True  # True: weights stationary, out^T [N, MC] on device


def _build(repeats=1, lookahead=5):
    nc = bacc.Bacc("TRN2", target_bir_lowering=False, debug=False, num_devices=NCORES)
    xT = nc.declare_dram_parameter("xT", [K, MC], F32, isOutput=False)
    wp = nc.declare_dram_parameter("wp", [K // 2, N], U8, isOutput=False)
    sc = nc.declare_dram_parameter("sc", [G, N], F32, isOutput=False)
    out = nc.declare_dram_parameter("out", [N, MC] if ORIENT_T else [MC, N],
                                    F32, isOutput=True)
    NBK = G // 2  # 16 dequant blocks per chunk (2 k-tiles each)

    with TileContext(nc) as tc:
        with tc.tile_pool(name="xt", bufs=1) as xt_pool, \
             tc.tile_pool(name="wpp", bufs=6) as wp_pool, \
             tc.tile_pool(name="nib", bufs=6) as nib_pool, \
             tc.tile_pool(name="deb", bufs=3) as deb_pool, \
             tc.tile_pool(name="wf", bufs=4) as wf_pool, \
             tc.tile_pool(name="scb", bufs=4) as sc_pool, \
             tc.tile_pool(name="ob", bufs=8) as out_pool, \
             tc.tile_pool(name="ps", bufs=8, space="PSUM") as psum_pool:

            xts = [None] * G  # resident activations, loaded just-in-time

            # flat schedule of dequant blocks across (rep, chunk, block)
            seq = []
            for rep in range(repeats):
                n0 = 0
                for ci, nsz in enumerate(N_CHUNKS):
                    for b in range(NBK):
                        seq.append((rep, ci, n0, nsz, b))
                    n0 += nsz

            wfts = {}    # flat index -> wft block tile [128, 2, nsz]
            ptiles = {}  # (rep, ci) -> list of psum tiles

            def produce(i):
                rep, ci, n0, nsz, b = seq[i]
                if rep == 0 and ci == 0:
                    for g in (2 * b, 2 * b + 1):
                        if xts[g] is None:
                            t = xt_pool.tile([128, MC], F32R, tag=f"xt{g}",
                                             name=f"xt{g}")
                            nc.scalar.dma_start(
                                out=t[:],
                                in_=xT[128 * g:128 * (g + 1), :].bitcast(F32R))
                            xts[g] = t
                # scale rows 2b (partitions 0..63) and 2b+1 (64..127)
                sct = sc_pool.tile([128, nsz], F32, name="sct")
                nc.scalar.dma_start(
                    out=sct[0:64, :],
                    in_=sc[2 * b:2 * b + 1, n0:n0 + nsz].to_broadcast([64, nsz]))
                nc.scalar.dma_start(
                    out=sct[64:128, :],
                    in_=sc[2 * b + 1:2 * b + 2, n0:n0 + nsz].to_broadcast([64, nsz]))
                # packed rows [128b, 128b+128), plain load
                wpt = wp_pool.tile([128, nsz], U8, name="wpt")
                nc.sync.dma_start(out=wpt[:],
                                  in_=wp[128 * b:128 * (b + 1), n0:n0 + nsz])
                # nibble extract into pair tile: [:,0,:] low, [:,1,:] high
                nib = nib_pool.tile([128, 2, nsz], U8, name="nib")
                nc.vector.tensor_scalar(out=nib[:, 0, :], in0=wpt[:],
                                        scalar1=15, scalar2=None,
                                        op0=mybir.AluOpType.bitwise_and)
                nc.vector.tensor_scalar(out=nib[:, 1, :], in0=wpt[:],
                                        scalar1=4, scalar2=None,
                                        op0=mybir.AluOpType.logical_shift_right)
                # debias + cast to f32 on ACT (both k-tiles in one op)
                deb = deb_pool.tile([128, 2, nsz], F32, name="deb")
                nc.scalar.activation(deb[:], nib[:],
                                     mybir.ActivationFunctionType.Copy,
                                     bias=-8.0, scale=1.0)
                wft = wf_pool.tile([128, 2, nsz], F32R, name="wft")
                nc.vector.tensor_tensor(
                    out=wft[:], in0=deb[:],
                    in1=sct[:].unsqueeze(1).to_broadcast([128, 2, nsz]),
                    op=mybir.AluOpType.mult)
                wfts[i] = wft

            MH = MC // 512  # moving m-chunks per k-tile (2)

            def consume(i):
                rep, ci, n0, nsz, b = seq[i]
                nt_cnt = nsz // 128
                if not ORIENT_T:
                    if b == 0:
                        ptiles[(rep, ci)] = [
                            psum_pool.tile([128, nsz], F32, name="ps", tag="ps")
                            for _ in range(MT)]
                    pts = ptiles[(rep, ci)]
                    wft = wfts.pop(i)
                    for t_half in range(2):
                        g = 2 * b + t_half
                        for m in range(MT):
                            nc.tensor.matmul(pts[m][:],
                                             xts[g][:, 128 * m:128 * (m + 1)],
                                             wft[:, t_half, :],
                                             start=(g == 0), stop=(g == G - 1))
                    if b == NBK - 1:
                        for m in range(MT):
                            ot = out_pool.tile([128, nsz], F32, name="ot")
                            if m % 2 == 0:
                                nc.vector.tensor_copy(ot[:], pts[m][:])
                            else:
                                nc.scalar.activation(ot[:], pts[m][:],
                                                     mybir.ActivationFunctionType.Copy)
                            nc.scalar.dma_start(
                                out=out[128 * m:128 * (m + 1), n0:n0 + nsz], in_=ot[:])
                        del ptiles[(rep, ci)]
                    return
                # ORIENT_T: lhsT = w (stationary, reused across 2 m-chunks),
                # rhs = xT (moving 512 wide), psum holds out^T [128 n, 512 m]
                if b == 0:
                    ptiles[(rep, ci)] = [
                        psum_pool.tile([128, 512], F32, name="ps", tag="ps")
                        for _ in range(nt_cnt * MH)]
                pts = ptiles[(rep, ci)]
                wft = wfts.pop(i)
                for t_half in range(2):
                    g = 2 * b + t_half
                    for nt in range(nt_cnt):
                        for h in range(MH):
                            nc.tensor.matmul(
                                pts[nt * MH + h][:],
                                wft[:, t_half, 128 * nt:128 * (nt + 1)],
                                xts[g][:, 512 * h:512 * (h + 1)],
                                start=(g == 0), stop=(g == G - 1))
                if b == NBK - 1:
                    for nt in range(nt_cnt):
                        for h in range(MH):
                            ot = out_pool.tile([128, 512], F32, name="ot")
                            if (nt + h) % 2 == 0:
                                nc.vector.tensor_copy(ot[:], pts[nt * MH + h][:])
                            else:
                                nc.scalar.activation(
                                    ot[:], pts[nt * MH + h][:],
                                    mybir.ActivationFunctionType.Copy)
                            nc.scalar.dma_start(
                                out=out[n0 + 128 * nt:n0 + 128 * (nt + 1),
                                        512 * h:512 * (h + 1)],
                                in_=ot[:])
                    del ptiles[(rep, ci)]

            for i in range(min(lookahead, len(seq))):
                produce(i)
            for i in range(len(seq)):
                if i + lookahead < len(seq):
                    produce(i + lookahead)
                consume(i)
    nc.compile()
    return nc


def kernel(x, col_indices, w_packed, scales, bias):
    global LAST_RESULTS, _CACHED_NC
    if _CACHED_NC is None:
        _CACHED_NC = _build()
    nc = _CACHED_NC

    x2 = np.ascontiguousarray(np.asarray(x, dtype=np.float32).reshape(M, K))
    perm = np.asarray(col_indices).astype(np.int64)[J_ORDER]
    wp_u8 = np.asarray(w_packed).astype(np.uint8)
    sc_f = np.ascontiguousarray(np.asarray(scales, dtype=np.float32))

    in_maps = []
    for c in range(NCORES):
        xTc = np.ascontiguousarray(x2[c * MC:(c + 1) * MC, perm].T)
        in_maps.append({"xT": xTc, "wp": wp_u8, "sc": sc_f})

    res = run_bass_kernel_spmd(nc, in_maps, list(range(NCORES)), trace=TRACE)
    LAST_RESULTS = res

    if ORIENT_T:
        out = np.concatenate(
            [np.ascontiguousarray(res.results[c]["out"].T) for c in range(NCORES)],
            axis=0)
    else:
        out = np.concatenate([res.results[c]["out"] for c in range(NCORES)], axis=0)
    out = out + np.asarray(bias, dtype=np.float32)[None, :]
    return np.ascontiguousarray(out.reshape(B, S, N).astype(np.float32))
